# revision 15
# baseline (speedup 1.0000x reference)
"""Trainium2 Bass kernel for nn_DeformConv2d (B=16, Cin=Cout=64, H=W=64, K=3).

Data-parallel over batch: 2 images per core on 8 cores; img0 on SBUF
partitions 0-63, img1 on 64-127.

v2 design (vs the 469us baseline):
  * Block-diagonal weights: one matmul serves both images (K=128), halving
    PE columns for the offset conv (lhsT [128,36]) and main conv
    (lhsT [128,128]).
  * Bilinear tent coefficients amap[img*9+j, n] = relu(1-|dy|)*relu(1-|dx|)
    computed compactly: merged [36,2048] Abs (ACT) -> min-sub tensor_scalar
    (DVE 4x mode) -> [18,2048] product (DVE).
  * No DVE adds at all: each of the 9 per-tap products amap_j * x_shift_j
    is fed straight to the PE as an accumulating matmul source against the
    block-diag W_kk (PSUM does the j-sum and the tap-sum).
  * Broadcast amap rows -> 128 partitions: js 0-6 via SWDGE SBUF->SBUF
    broadcast DMA (descriptor round-robin spreads all 16 DMA engines;
    two issues per tap: a 4-j quad and a 3-j trio); js 7-8 via PE ones-mask
    matmul + ACT PSUM evacuation.
  * One product per tap runs on the Pool engine (gpsimd tensor_tensor) to
    offload DVE.
  * offs / amap never round-trip DRAM; output store via SWDGE for spread.

kernel() accepts FULL inputs and returns the FULL [16,64,64,64] output.
"""

import numpy as np
from contextlib import ExitStack

N_CORES = 8
B, CIN, COUT, H, W = 16, 64, 64, 64, 64
KK = 9  # 3x3 taps
HW = H * W  # 4096
PADR, PADC = 2, 2
HP, WP = H + 2 * PADR, W + 2 * PADC  # 68, 68
IMG_PER_CORE = B // N_CORES  # 2
HALF = HW // 2  # 2048

# work split knobs
QUAD_JS = (0, 1, 2, 3)   # SWDGE quad broadcast
TRIO_JS = (4, 5, 6)      # SWDGE trio broadcast
PE_JS = (7, 8)           # PE ones-mask broadcast
POOL_MULT_JS = ()        # products computed on Pool engine

_cache = {}


def _build_program():
    import concourse.bass as bass  # noqa: F401
    import concourse.mybir as mybir
    import concourse.tile as tile
    from concourse import bacc

    fp16 = mybir.dt.float16
    f32 = mybir.dt.float32
    AOp = mybir.AluOpType
    AF = mybir.ActivationFunctionType

    nc = bacc.Bacc("TRN2", target_bir_lowering=False, debug=False,
                   num_devices=N_CORES)

    xp_ext = nc.declare_dram_parameter("xp", [128, HP * WP], fp16, isOutput=False)
    woff_ext = nc.declare_dram_parameter("woff", [128, KK * 36], fp16, isOutput=False)
    wdcn_ext = nc.declare_dram_parameter("wdcn", [128, KK * 128], fp16, isOutput=False)
    boff_ext = nc.declare_dram_parameter("boff", [36, 1], f32, isOutput=False)
    bdcn_ext = nc.declare_dram_parameter("bdcn", [128, 1], f32, isOutput=False)
    ones2_ext = nc.declare_dram_parameter("ones2", [18, len(PE_JS) * 128], fp16,
                                          isOutput=False)
    tby_ext = nc.declare_dram_parameter("tby", [18, 1], f32, isOutput=False)
    tbx_ext = nc.declare_dram_parameter("tbx", [18, 1], f32, isOutput=False)
    out_ext = nc.declare_dram_parameter("out", [128, HW], f32, isOutput=True)

    # amap bounce buffer: rows = img, cols = (kk, h, j, n) so j-runs are
    # contiguous per (kk, h) and a single broadcast read serves several js
    amap_dram = nc.dram_tensor("amap_dram", [2, KK * 2 * KK * HALF], fp16)
    offs_dram = nc.dram_tensor("offs_dram", [36, HW], fp16)

    with tile.TileContext(nc) as tc, ExitStack() as ctx:
        pool = ctx.enter_context(tc.tile_pool(name="sbuf", bufs=1))
        tpool = ctx.enter_context(tc.tile_pool(name="tents", bufs=2))
        apool = ctx.enter_context(tc.tile_pool(name="amaps", bufs=3))
        abuf = ctx.enter_context(tc.tile_pool(name="areps", bufs=2))
        dbuf = ctx.enter_context(tc.tile_pool(name="prods", bufs=6))
        ppool = ctx.enter_context(tc.tile_pool(name="psum", bufs=1, space="PSUM"))
        pbc = ctx.enter_context(tc.tile_pool(name="psumbc", bufs=3, space="PSUM"))
        poff = ctx.enter_context(tc.tile_pool(name="psumoff", bufs=1, space="PSUM"))

        # ---- inputs ----
        xp = pool.tile([128, HP * WP], fp16)
        nc.gpsimd.dma_start(xp[:], xp_ext[:])
        xp3 = xp[:].rearrange("p (r c) -> p r c", c=WP)  # [128, 68, 68]

        woff = pool.tile([128, KK * 36], fp16)
        nc.sync.dma_start(woff[:], woff_ext[:])
        wdcn = pool.tile([128, KK * 128], fp16)
        nc.sync.dma_start(wdcn[:], wdcn_ext[:])
        boff = pool.tile([36, 1], f32)
        nc.sync.dma_start(boff[:], boff_ext[:])
        bdcn = pool.tile([128, 1], f32)
        nc.sync.dma_start(bdcn[:], bdcn_ext[:])
        ones2 = pool.tile([18, len(PE_JS) * 128], fp16)
        nc.sync.dma_start(ones2[:], ones2_ext[:])
        tby = pool.tile([18, 1], f32)
        nc.sync.dma_start(tby[:], tby_ext[:])
        tbx = pool.tile([18, 1], f32)
        nc.sync.dma_start(tbx[:], tbx_ext[:])

        offs_sb = pool.tile([36, HW], fp16)
        offs3 = offs_dram[:].rearrange("(i c) n -> i c n", i=2)  # [2, 18, HW]
        out_sb = pool.tile([128, HW], f32)
        ad5 = amap_dram[:].rearrange(
            "p (k h j n) -> p k h j n", k=KK, h=2, j=KK
        )
        ad4 = amap_dram[:].rearrange(
            "p (k h m) -> p k h m", k=KK, h=2
        )  # [2, 9, 2, KK*HALF]

        # ---- offset conv: 8 chunks of 512 cols, block-diag lhsT [128,36] ----
        def emit_offconv_chunk(q):
            ps = poff.tile([36, 512], f32, tag="off")
            for kk in range(KK):
                ky, kx = kk // 3, kk % 3
                rhs = xp3[
                    :,
                    (PADR - 1 + ky + 8 * q) : (PADR - 1 + ky + 8 * q + 8),
                    (PADC - 1 + kx) : (PADC - 1 + kx + W),
                ]  # [128, 8, 64] -> 512 cols
                nc.tensor.matmul(
                    ps[:],
                    woff[:, kk * 36 : (kk + 1) * 36],
                    rhs,
                    start=(kk == 0),
                    stop=(kk == KK - 1),
                )
            qs = slice(q * 512, (q + 1) * 512)
            nc.scalar.activation(
                out=offs_sb[:, qs], in_=ps[:],
                func=AF.Identity, bias=boff[:],
            )
            nc.sync.dma_start(offs_dram[:, qs], offs_sb[:, qs])

        # ---- per-tap pipeline pieces ----
        tin_tiles = {}
        amap_tiles = {}
        quad_tiles = {}
        trio_tiles = {}
        arep_tiles = {}

        def emit_tin(kk, h):
            # tin_y/tin_x rows: img*9 + j  <- offs row img*18 + 2kk + ax
            tin_y = tpool.tile([18, HALF], fp16, tag="tiny")
            tin_x = tpool.tile([18, HALF], fp16, tag="tinx")
            tin_tiles[(kk, h)] = (tin_y, tin_x)
            hs = slice(h * HALF, (h + 1) * HALF)
            for ax, tin, eng in ((0, tin_y, nc.sync), (1, tin_x, nc.scalar)):
                src = offs3[:, 2 * kk + ax : 2 * kk + ax + 1, hs].broadcast_to(
                    [2, KK, HALF]
                )
                eng.dma_start(tin[:], src)

        def emit_tents(kk, h):
            # u = |tin + tb|; t = min(u,1)-1; amap = t_y * t_x
            tin_y, tin_x = tin_tiles.pop((kk, h))
            uy = tpool.tile([18, HALF], fp16, tag="uy")
            nc.scalar.activation(out=uy[:], in_=tin_y[:], func=AF.Abs, bias=tby[:])
            ux = tpool.tile([18, HALF], fp16, tag="ux")
            nc.scalar.activation(out=ux[:], in_=tin_x[:], func=AF.Abs, bias=tbx[:])
            ty = tpool.tile([18, HALF], fp16, tag="ty")
            nc.vector.tensor_scalar(
                out=ty[:], in0=uy[:], scalar1=1.0, scalar2=1.0,
                op0=AOp.min, op1=AOp.subtract,
            )
            tx = tpool.tile([18, HALF], fp16, tag="tx")
            nc.vector.tensor_scalar(
                out=tx[:], in0=ux[:], scalar1=1.0, scalar2=1.0,
                op0=AOp.min, op1=AOp.subtract,
            )
            amap = apool.tile([18, HALF], fp16, tag="amap")
            amap_tiles[(kk, h)] = amap
            nc.vector.tensor_tensor(
                out=amap[:], in0=ty[:], in1=tx[:], op=AOp.mult
            )
            # bounce the js served by SWDGE broadcast through DRAM
            nc.sync.dma_start(ad5[:, kk, h, :, :], amap[:])

        def emit_bcast_dma(kk, h):
            # SWDGE broadcast of js QUAD_JS and TRIO_JS from amap rows
            nq = len(QUAD_JS)
            quad = abuf.tile([128, nq * HALF], fp16, tag="quad")
            quad_tiles[(kk, h)] = quad
            src = (
                ad4[
                    :, kk, h,
                    QUAD_JS[0] * HALF : (QUAD_JS[0] + nq) * HALF,
                ]
                .unsqueeze(1)
                .broadcast_to([2, 64, nq * HALF])
            )
            nc.gpsimd.dma_start(quad[:], src)
            nt = len(TRIO_JS)
            trio = abuf.tile([128, nt * HALF], fp16, tag="trio")
            trio_tiles[(kk, h)] = trio
            src = (
                ad4[
                    :, kk, h,
                    TRIO_JS[0] * HALF : (TRIO_JS[0] + nt) * HALF,
                ]
                .unsqueeze(1)
                .broadcast_to([2, 64, nt * HALF])
            )
            nc.gpsimd.dma_start(trio[:], src)

        def emit_bcast_pe(kk, h):
            # PE ones-mask broadcast for PE_JS, ACT evacuates PSUM -> fp16
            amap = amap_tiles[(kk, h)]
            for s, j in enumerate(PE_JS):
                arep = abuf.tile([128, HALF], fp16, tag=f"arep{s}")
                arep_tiles[(kk, h, j)] = arep
                for q in range(4):
                    psb = pbc.tile([128, 512], f32, tag="bc")
                    nc.tensor.matmul(
                        psb[:],
                        ones2[:, s * 128 : (s + 1) * 128],
                        amap[:, q * 512 : (q + 1) * 512],
                        start=True,
                        stop=True,
                    )
                    nc.scalar.activation(
                        out=arep[:, q * 512 : (q + 1) * 512], in_=psb[:],
                        func=AF.Identity,
                    )

        def xwin(kk, j, h):
            ky, kx = kk // 3, kk % 3
            dy, dx = j // 3 - 1, j % 3 - 1
            return xp3[
                :,
                (PADR - 1 + ky + dy + 32 * h) : (PADR - 1 + ky + dy + 32 * h + 32),
                (PADC - 1 + kx + dx) : (PADC - 1 + kx + dx + W),
            ]  # [128, 32, 64] -> 2048 cols

        def arep_view(kk, h, j):
            if j in QUAD_JS:
                qd = quad_tiles[(kk, h)]
                s = QUAD_JS.index(j)
                return qd[:, s * HALF : (s + 1) * HALF]
            if j in TRIO_JS:
                tr = trio_tiles[(kk, h)]
                s = TRIO_JS.index(j)
                return tr[:, s * HALF : (s + 1) * HALF]
            return arep_tiles[(kk, h, j)][:]

        # ---- main loop ----
        # prologue: first-half offset conv, first tap tents + broadcasts
        for q in range(4):
            emit_offconv_chunk(q)
        emit_tin(0, 0)
        emit_tents(0, 0)
        emit_bcast_dma(0, 0)
        emit_bcast_pe(0, 0)
        emit_tin(1, 0)

        for h in range(2):
            ps_main = ppool.tile([128, HALF], f32, tag="big")
            for kk in range(KK):
                # lookahead: tents/broadcasts for next tap, offset chunks in h0
                nkk, nh = (kk + 1, h) if kk + 1 < KK else (0, h + 1)
                if nh < 2:
                    emit_tents(nkk, nh)
                    emit_bcast_dma(nkk, nh)
                    emit_bcast_pe(nkk, nh)
                    # prefetch tin one more tap ahead
                    nnkk, nnh = (nkk + 1, nh) if nkk + 1 < KK else (0, nh + 1)
                    if nnh < 2:
                        emit_tin(nnkk, nnh)
                if h == 0 and kk < 4:
                    emit_offconv_chunk(4 + kk)

                amap_tiles.pop((kk, h))
                # 9 products -> 9 accumulating matmul sources
                for ji, j in enumerate(
                    QUAD_JS + TRIO_JS + PE_JS
                ):
                    prod = dbuf.tile([128, HALF], fp16, tag="prod")
                    eng = nc.gpsimd if j in POOL_MULT_JS else nc.vector
                    eng.tensor_tensor(
                        out=prod[:].rearrange("p (a b) -> p a b", b=W),
                        in0=xwin(kk, j, h),
                        in1=arep_view(kk, h, j).rearrange("p (a b) -> p a b", b=W),
                        op=AOp.mult,
                    )
                    for t in range(4):
                        nc.tensor.matmul(
                            ps_main[:, t * 512 : (t + 1) * 512],
                            wdcn[:, kk * 128 : (kk + 1) * 128],
                            prod[:, t * 512 : (t + 1) * 512],
                            start=(kk == 0 and ji == 0),
                            stop=(kk == KK - 1 and ji == KK - 1),
                        )
                quad_tiles.pop((kk, h))
                trio_tiles.pop((kk, h))
                for j in PE_JS:
                    arep_tiles.pop((kk, h, j))
            # evacuate with bias, store via SWDGE (descriptor spread)
            hs = slice(h * HALF, (h + 1) * HALF)
            nc.scalar.activation(
                out=out_sb[:, hs], in_=ps_main[:],
                func=AF.Identity, bias=bdcn[:],
            )
            nc.sync.dma_start(out_ext[:, hs], out_sb[:, hs])

    nc.compile()
    return nc


def _host_prep(x, w_off, b_off, w_dcn, b_dcn):
    """Per-core input maps. numpy layout/dtype prep only."""
    fp16 = np.float16
    x = np.asarray(x, dtype=np.float32)
    w_off = np.asarray(w_off, dtype=np.float32)
    b_off = np.asarray(b_off, dtype=np.float32)
    w_dcn = np.asarray(w_dcn, dtype=np.float32)
    b_dcn = np.asarray(b_dcn, dtype=np.float32)

    # block-diag lhsT: [128, KK*36] and [128, KK*128]
    w4o = w_off.transpose(2, 3, 1, 0).reshape(KK, CIN, 18)  # [kk, c, m]
    woff2 = np.zeros((128, KK, 36), np.float32)
    woff2[0:64, :, 0:18] = w4o.transpose(1, 0, 2)
    woff2[64:128, :, 18:36] = w4o.transpose(1, 0, 2)
    woff2 = woff2.reshape(128, KK * 36).astype(fp16)

    w4d = w_dcn.transpose(2, 3, 1, 0).reshape(KK, CIN, COUT)  # [kk, c, o]
    wdcn2 = np.zeros((128, KK, 128), np.float32)
    wdcn2[0:64, :, 0:64] = w4d.transpose(1, 0, 2)
    wdcn2[64:128, :, 64:128] = w4d.transpose(1, 0, 2)
    wdcn2 = wdcn2.reshape(128, KK * 128).astype(fp16)

    boff2 = np.zeros((36, 1), np.float32)
    boff2[0:18, 0] = b_off
    boff2[18:36, 0] = b_off
    bdcn2 = np.tile(b_dcn, IMG_PER_CORE).reshape(128, 1).astype(np.float32)

    # ones-mask lhsT for PE broadcasts: rows [18], cols [s*128 + m]
    ones2 = np.zeros((18, len(PE_JS) * 128), fp16)
    for s, j in enumerate(PE_JS):
        for m in range(128):
            ones2[(m // 64) * KK + j, s * 128 + m] = 1.0

    # tent biases: row img*9 + j -> -(dy) / -(dx)
    tby = np.zeros((18, 1), np.float32)
    tbx = np.zeros((18, 1), np.float32)
    for j in range(KK):
        for img in range(2):
            tby[img * KK + j, 0] = -(j // 3 - 1)
            tbx[img * KK + j, 0] = -(j % 3 - 1)

    shared = {
        "woff": woff2,
        "wdcn": wdcn2,
        "boff": boff2,
        "bdcn": bdcn2,
        "ones2": ones2,
        "tby": tby,
        "tbx": tbx,
    }
    in_maps = []
    for core in range(N_CORES):
        imgs = x[core * IMG_PER_CORE : (core + 1) * IMG_PER_CORE]
        xp = np.zeros((IMG_PER_CORE, CIN, HP, WP), np.float32)
        xp[:, :, PADR : PADR + H, PADC : PADC + W] = imgs
        m = {"xp": xp.reshape(128, HP * WP).astype(fp16)}
        m.update(shared)
        in_maps.append(m)
    return in_maps


def kernel(x, w_off, b_off, w_dcn, b_dcn, _trace=False):
    from concourse.bass_utils import run_bass_kernel_spmd

    if "nc" not in _cache:
        _cache["nc"] = _build_program()
    nc = _cache["nc"]

    in_maps = _host_prep(x, w_off, b_off, w_dcn, b_dcn)
    res = run_bass_kernel_spmd(nc, in_maps, list(range(N_CORES)), trace=_trace)
    _cache["last_result"] = res

    out = np.empty((B, COUT, H, W), np.float32)
    for core in range(N_CORES):
        o = np.asarray(res.results[core]["out"], dtype=np.float32)
        out[core * IMG_PER_CORE : (core + 1) * IMG_PER_CORE] = o.reshape(
            IMG_PER_CORE, COUT, H, W
        )
    return out
